# revision 11
# baseline (speedup 1.0000x reference)
"""Trainium2 Bass kernel for nn_BidirectionalTemporalAttention.

Reference computation (B=2, T=16, F=128, D=1024, N=T*F=2048):
  xf = x.reshape(B, N, D)
  lookback branch: 8 heads, E=64, causal mask (keep k <= q)
  lookahead branch: 8 heads, anti-causal (keep k >= q)
  o = concat([o_lb, o_la], heads) -> (B, 16, N, 64) -> RAW reshape (B, N, D)
  out = o @ Wo^T -> (B, T, F, D)

The raw reshape means out row r = h*128 + g depends only on head h (tokens
16g..16g+15 of that head).  So with 4 heads per core each core's 512 output
rows are fully local: no collectives, the host just concatenates row slices.

Sharding over 8 cores: (batch b in 2) x (group in [lb0-3, lb4-7, la0-3, la4-7]).
Lookahead cores receive the token-reversed sequence so one SPMD causal program
serves all cores; their outputs are un-reversed on the host (row reversal
within each 128-row head block, plus a j-group reversal folded into Wo).

Per-core kernel layout choices:
  - S^T blocks [k(128 part), q(512 free)] so softmax-denominator and PV both
    contract over k on the partition axis.
  - exp has no max-subtraction (scores are O(10), safe in fp32); softmax
    denominator comes free as a ones-column appended to V in the PV matmul.
  - attention output written into O2T [128=(n%2)*64+e, n//2]; its strided
    views O2T[:, m::8] are exactly the K=128 lhsT tiles the out-projection
    needs under the reference's raw (H,N,E)->(N,D) reshape.
  - all matmul operands are float32r (1 cycle/row on the PE at free>=256,
    ~1.5e-4 relative rounding vs 4 cycles/row for exact fp32).
"""

import sys

if "/opt/trn_rl_repo" not in sys.path:
    sys.path.insert(0, "/opt/trn_rl_repo")

import numpy as np

import concourse.bass as bass  # noqa: F401
import concourse.mybir as mybir
import concourse.tile as tile
from concourse import bacc
from concourse import bass_utils as _bu
from concourse.bass_utils import run_bass_kernel_spmd

# LDWEIGHTS of fp32 weights serializes with matmuls and costs ~35% of PE time
# on this kernel; walrus's ldw optimization (background weight-buffer loads)
# is disabled by default in this toolchain. Flip just that flag.
_ENABLE_LDW_OPT = True
_orig_run_command = _bu.run_command


def _patched_run_command(cmd, *a, **kw):
    if _ENABLE_LDW_OPT and isinstance(cmd, list):
        cmd = [
            "--enable-ldw-opt=true" if c == "--enable-ldw-opt=false" else c
            for c in cmd
        ]
    return _orig_run_command(cmd, *a, **kw)


_bu.run_command = _patched_run_command

F32 = mybir.dt.float32
F32R = mybir.dt.float32r
EXP = mybir.ActivationFunctionType.Exp

N = 2048  # tokens per batch
D = 1024  # embed dim
E = 64  # head dim
HPC = 4  # heads per core
NQB = 4  # query blocks of 512
NKB = 16  # key blocks of 128
NDB = 8  # d blocks of 128
SCALE = 0.125  # 1/sqrt(E)

_CACHE = {}


def build_nc():
    nc = bacc.Bacc("TRN2", target_bir_lowering=False, debug=False)

    xt_d = nc.dram_tensor("xt", [D, N], F32R, kind="ExternalInput").ap()
    wq_d = nc.dram_tensor("wq", [128, NDB, 256], F32R, kind="ExternalInput").ap()
    wk_d = nc.dram_tensor("wk", [128, NDB, 256], F32R, kind="ExternalInput").ap()
    wv_d = nc.dram_tensor("wv", [128, NDB, 256], F32R, kind="ExternalInput").ap()
    wo_d = nc.dram_tensor("wo", [128, 8, D], F32R, kind="ExternalInput").ap()
    mk_d = nc.dram_tensor("mk", [128, 4, 512], F32R, kind="ExternalInput").ap()
    out_d = nc.dram_tensor("out", [512, D], F32, kind="ExternalOutput").ap()

    with tile.TileContext(nc) as tc:
        with (
            tc.tile_pool(name="w", bufs=1) as wp,
            tc.tile_pool(name="xp", bufs=8) as xp,
            tc.tile_pool(name="qkv", bufs=1) as qkvp,
            tc.tile_pool(name="pt", bufs=4) as ptp,
            tc.tile_pool(name="o2t", bufs=4) as o2tp,
            tc.tile_pool(name="ob", bufs=3) as obp,
            tc.tile_pool(name="rc", bufs=3) as rcp,
            tc.tile_pool(name="pss", bufs=2, space="PSUM") as pss,
            tc.tile_pool(name="psq", bufs=2, space="PSUM") as psq,
        ):
            # --- weights / constants ---
            wq_sb = wp.tile([128, NDB, 256], F32R, tag="wq")
            wk_sb = wp.tile([128, NDB, 256], F32R, tag="wk")
            wv_sb = wp.tile([128, NDB, 256], F32R, tag="wv")
            wo_sb = wp.tile([128, 8, D], F32R, tag="wo")
            mk_sb = wp.tile([128, 4, 512], F32R, tag="mk")
            ones_sb = wp.tile([1, 64], F32R, tag="ones")
            nc.sync.dma_start(wq_sb[:], wq_d)
            nc.sync.dma_start(wk_sb[:], wk_d)
            nc.sync.dma_start(wv_sb[:], wv_d)
            nc.sync.dma_start(wo_sb[:], wo_d)
            nc.sync.dma_start(mk_sb[:], mk_d)
            ones_f32 = wp.tile([128, 64], F32, tag="ones_f32")
            nc.vector.memset(ones_f32[:], 1.0)
            nc.vector.tensor_copy(ones_sb[:], ones_f32[0:1, :])

            # persistent Q^T / K^T / V(+ones column)
            qt = qkvp.tile([128, 2, N], F32R, tag="qt")  # [(2h)*64e, pair, n]
            kt = qkvp.tile([128, 2, N], F32R, tag="kt")
            vt = qkvp.tile([128, NKB, HPC, 65], F32R, tag="vt")  # [k, kb, h, e|1]
            nc.vector.tensor_copy(
                vt[:, :, :, 64],
                ones_f32[:, 0 : NKB * HPC].rearrange("p (a b) -> p a b", a=NKB),
            )

            # --- phase 1: QKV projections, in two n-halves ---
            for nh in range(2):
                xts = []
                for do in range(NDB):
                    xtile = xp.tile([128, 1024], F32R, tag="xt")
                    nc.sync.dma_start(
                        xtile[:],
                        xt_d[do * 128 : (do + 1) * 128, nh * 1024 : (nh + 1) * 1024],
                    )
                    xts.append(xtile)
                # Q^T and K^T: out [128=(2 heads x 64e), 512 n]
                for w_sb, dst in ((wq_sb, qt), (wk_sb, kt)):
                    for mg in range(2):
                        for qbl in range(2):
                            qb = nh * 2 + qbl
                            ps = pss.tile([128, 512], F32, tag="sc")
                            for do in range(NDB):
                                nc.tensor.matmul(
                                    ps[:],
                                    w_sb[:, do, mg * 128 : (mg + 1) * 128],
                                    xts[do][:, qbl * 512 : (qbl + 1) * 512],
                                    start=(do == 0),
                                    stop=(do == NDB - 1),
                                )
                            nc.vector.tensor_copy(
                                dst[:, mg, qb * 512 : (qb + 1) * 512], ps[:]
                            )
                # V: out [128 k, 256=(4 heads x 64e)]
                for kbl in range(8):
                    kb = nh * 8 + kbl
                    ps = pss.tile([128, 512], F32, tag="sc")
                    nps = ps[:, 0:256]
                    for do in range(NDB):
                        nc.tensor.matmul(
                            nps,
                            xts[do][:, kbl * 128 : (kbl + 1) * 128],
                            wv_sb[:, do, :],
                            start=(do == 0),
                            stop=(do == NDB - 1),
                        )
                    nc.vector.tensor_copy(
                        vt[:, kb, :, 0:64], nps.rearrange("p (h e) -> p h e", h=HPC)
                    )

            # --- phase 2: attention + out-proj, per head pair ---
            for pr in range(2):
                o2t_h = [o2tp.tile([128, N // 2], F32R, tag="o2", name=f"o2t_{pr}_{i}") for i in range(2)]
                for qb in range(NQB):
                    nkb = 4 * qb + 4  # kept key blocks (causal)
                    o_ps = [pss.tile([128, 512], F32, tag="ov", name=f"ov_{pr}_{qb}_{i}") for i in range(2)]
                    for kb in range(nkb):
                        # S^T duo: both heads of the pair for this key block
                        qd = psq.tile([128, 2, 512], F32, tag="qd")
                        for h in range(2):
                            nc.tensor.matmul(
                                qd[:, h, :],
                                kt[64 * h : 64 * (h + 1), pr, kb * 128 : (kb + 1) * 128],
                                qt[64 * h : 64 * (h + 1), pr, qb * 512 : (qb + 1) * 512],
                                start=True,
                                stop=True,
                            )
                        pt_t = ptp.tile([128, 2, 512], F32R, tag="pt")
                        nc.scalar.activation(pt_t[:], qd[:], EXP, scale=SCALE)
                        dg = kb - 4 * qb  # diagonal mask pattern (0..3) if >= 0
                        if dg >= 0:
                            for h in range(2):
                                nc.vector.tensor_mul(
                                    pt_t[:, h, :], pt_t[:, h, :], mk_sb[:, dg, :]
                                )
                        for h in range(2):
                            nc.tensor.matmul(
                                o_ps[h][0:65, :],
                                vt[:, kb, 2 * pr + h, :],
                                pt_t[:, h, :],
                                start=(kb == 0),
                                stop=(kb == nkb - 1),
                            )
                    # normalize into O2T [ (n%2)*64+e , n//2 ]
                    for h in range(2):
                        rec = rcp.tile([1, 512], F32R, tag="rec")
                        with nc.allow_low_precision(reason="f32r is 4-byte"):
                            nc.vector.reciprocal(rec[:], o_ps[h][64:65, :])
                        brp = pss.tile([128, 512], F32, tag="sc")
                        nc.tensor.matmul(
                            brp[0:64, :], ones_sb[:], rec[:], start=True, stop=True
                        )
                        brs = rcp.tile([64, 512], F32R, tag="brs")
                        nc.vector.tensor_copy(brs[:], brp[0:64, :])
                        for par in range(2):
                            nc.vector.tensor_mul(
                                o2t_h[h][
                                    64 * par : 64 * par + 64,
                                    256 * qb : 256 * (qb + 1),
                                ],
                                o_ps[h][0:64, par::2],
                                brs[:, par::2],
                            )
                # out-projection: out rows for head hl = 2*pr + h
                for h in range(2):
                    hl = 2 * pr + h
                    for oh in range(2):
                        op = pss.tile([128, 512], F32, tag="sc")
                        for m in range(8):
                            nc.tensor.matmul(
                                op[:],
                                o2t_h[h][:, m::8],
                                wo_sb[:, m, oh * 512 : (oh + 1) * 512],
                                start=(m == 0),
                                stop=(m == 7),
                            )
                        osb = obp.tile([128, 512], F32, tag="ob")
                        nc.vector.tensor_copy(osb[:], op[:])
                        nc.sync.dma_start(
                            out_d[hl * 128 : (hl + 1) * 128, oh * 512 : (oh + 1) * 512],
                            osb[:],
                        )

    nc.compile()
    return nc


def _get_nc():
    if "nc" not in _CACHE:
        _CACHE["nc"] = build_nc()
    return _CACHE["nc"]


def _prep_w(wg):
    """(4, 64, 1024) per-head weights -> [128, 8, 256] SBUF lhsT layout."""
    # WT[d, f=(h*64+e)] = wg[h, e, d]; block d = do*128 + p -> [p, do, f]
    wt = wg.transpose(2, 0, 1).reshape(D, 256)
    return np.ascontiguousarray(wt.reshape(NDB, 128, 256).transpose(1, 0, 2))


def _prep_wo(wot):
    """WoT (1024, 1024) [c, o] -> [128, 8, 1024] with c = 128*m + p."""
    return np.ascontiguousarray(wot.reshape(8, 128, D).transpose(1, 0, 2))


def make_in_maps(x, Wq_lb, Wk_lb, Wv_lb, Wq_la, Wk_la, Wv_la, Wo):
    B = x.shape[0]
    xf = np.asarray(x, np.float32).reshape(B, N, D)
    wot = np.ascontiguousarray(np.asarray(Wo, np.float32).T)  # [c, o]
    wot_rev = np.ascontiguousarray(wot.reshape(16, 64, D)[::-1].reshape(D, D))
    wo_maps = {False: _prep_wo(wot), True: _prep_wo(wot_rev)}

    kp = np.arange(128, dtype=np.int64)[:, None]
    qf = np.arange(512, dtype=np.int64)[None, :]
    mk = np.stack(
        [(qf >= kp + 128 * dg).astype(np.float32) for dg in range(4)], axis=0
    )
    mk = np.ascontiguousarray(mk.transpose(1, 0, 2))  # [128, 4, 512]

    xts = {}
    for b in range(B):
        xts[(b, False)] = np.ascontiguousarray(xf[b].T)
        xts[(b, True)] = np.ascontiguousarray(xf[b][::-1].T)

    wsel = {
        False: (np.asarray(Wq_lb, np.float32), np.asarray(Wk_lb, np.float32),
                np.asarray(Wv_lb, np.float32)),
        True: (np.asarray(Wq_la, np.float32), np.asarray(Wk_la, np.float32),
               np.asarray(Wv_la, np.float32)),
    }
    wcache = {}
    in_maps = []
    for c in range(8):
        b, grp = divmod(c, 4)
        la = grp >= 2
        half = grp % 2
        key = (la, half)
        if key not in wcache:
            wq, wk, wv = wsel[la]
            sl = slice(half * 4, half * 4 + 4)
            wcache[key] = (_prep_w(wq[sl]), _prep_w(wk[sl]), _prep_w(wv[sl]))
        pwq, pwk, pwv = wcache[key]
        in_maps.append(
            {
                "xt": xts[(b, la)],
                "wq": pwq,
                "wk": pwk,
                "wv": pwv,
                "wo": wo_maps[la],
                "mk": mk,
            }
        )
    return in_maps


def assemble(results, B=2):
    out = np.empty((B, N, D), np.float32)
    for c in range(8):
        b, grp = divmod(c, 4)
        part = np.asarray(results[c]["out"])  # (512, 1024)
        if grp >= 2:  # lookahead: un-reverse rows within each head block
            part = part.reshape(HPC, 128, D)[:, ::-1].reshape(512, D)
        out[b, grp * 512 : (grp + 1) * 512] = part
    return out


def kernel(x, Wq_lb, Wk_lb, Wv_lb, Wq_la, Wk_la, Wv_la, Wo):
    nc = _get_nc()
    in_maps = make_in_maps(x, Wq_lb, Wk_lb, Wv_lb, Wq_la, Wk_la, Wv_la, Wo)
    res = run_bass_kernel_spmd(nc, in_maps, list(range(8)))
    B, T, F_, D_ = x.shape
    return assemble(res.results, B).reshape(B, T, F_, D_)


# revision 14
# speedup vs baseline: 1.1870x; 1.1870x over previous
"""Trainium2 Bass kernel for nn_BidirectionalTemporalAttention.

Reference computation (B=2, T=16, F=128, D=1024, N=T*F=2048):
  xf = x.reshape(B, N, D)
  lookback branch: 8 heads, E=64, causal mask (keep k <= q)
  lookahead branch: 8 heads, anti-causal (keep k >= q)
  o = concat([o_lb, o_la], heads) -> (B, 16, N, 64) -> RAW reshape (B, N, D)
  out = o @ Wo^T -> (B, T, F, D)

The raw reshape means out row r = h*128 + g depends only on head h (tokens
16g..16g+15 of that head).  So with 4 heads per core each core's 512 output
rows are fully local: no collectives, the host just concatenates row slices.

Sharding over 8 cores: (batch b in 2) x (group in [lb0-3, lb4-7, la0-3, la4-7]).
Lookahead cores receive the token-reversed sequence so one SPMD causal program
serves all cores; their outputs are un-reversed on the host (row reversal
within each 128-row head block, plus a j-group reversal folded into Wo).

Per-core kernel layout choices:
  - S^T blocks [k(128 part), q(512 free)] so softmax-denominator and PV both
    contract over k on the partition axis.
  - exp has no max-subtraction (scores are O(10), safe in fp32); softmax
    denominator comes free as a ones-column appended to V in the PV matmul.
  - attention output written into O2T [128=(n%2)*64+e, n//2]; its strided
    views O2T[:, m::8] are exactly the K=128 lhsT tiles the out-projection
    needs under the reference's raw (H,N,E)->(N,D) reshape.
  - all matmul operands are float32r (1 cycle/row on the PE at free>=256,
    ~1.5e-4 relative rounding vs 4 cycles/row for exact fp32).
"""

import sys

if "/opt/trn_rl_repo" not in sys.path:
    sys.path.insert(0, "/opt/trn_rl_repo")

import numpy as np

import concourse.bass as bass  # noqa: F401
import concourse.mybir as mybir
import concourse.tile as tile
from concourse import bacc
from concourse import bass_utils as _bu
from concourse.bass_utils import run_bass_kernel_spmd

# LDWEIGHTS of fp32 weights serializes with matmuls and costs ~35% of PE time
# on this kernel; walrus's ldw optimization (background weight-buffer loads)
# is disabled by default in this toolchain. Flip just that flag.
_ENABLE_LDW_OPT = False
_orig_run_command = _bu.run_command


def _patched_run_command(cmd, *a, **kw):
    if _ENABLE_LDW_OPT and isinstance(cmd, list):
        cmd = [
            "--enable-ldw-opt=true" if c == "--enable-ldw-opt=false" else c
            for c in cmd
        ]
    return _orig_run_command(cmd, *a, **kw)


_bu.run_command = _patched_run_command

F32 = mybir.dt.float32
F32R = mybir.dt.float32r
EXP = mybir.ActivationFunctionType.Exp

N = 2048  # tokens per batch
D = 1024  # embed dim
E = 64  # head dim
HPC = 4  # heads per core
NQB = 4  # query blocks of 512
NKB = 16  # key blocks of 128
NDB = 8  # d blocks of 128
SCALE = 0.125  # 1/sqrt(E)

_CACHE = {}


def build_nc():
    nc = bacc.Bacc("TRN2", target_bir_lowering=False, debug=False)

    xt_d = nc.dram_tensor("xt", [D, N], F32R, kind="ExternalInput").ap()
    wq_d = nc.dram_tensor("wq", [128, NDB, 256], F32R, kind="ExternalInput").ap()
    wk_d = nc.dram_tensor("wk", [128, NDB, 256], F32R, kind="ExternalInput").ap()
    wv_d = nc.dram_tensor("wv", [128, NDB, 256], F32R, kind="ExternalInput").ap()
    wo_d = nc.dram_tensor("wo", [128, 8, D], F32R, kind="ExternalInput").ap()
    mk_d = nc.dram_tensor("mk", [128, 4, 512], F32R, kind="ExternalInput").ap()
    out_d = nc.dram_tensor("out", [512, D], F32, kind="ExternalOutput").ap()

    with tile.TileContext(nc) as tc:
        with (
            tc.tile_pool(name="w", bufs=1) as wp,
            tc.tile_pool(name="xp", bufs=8) as xp,
            tc.tile_pool(name="qkv", bufs=1) as qkvp,
            tc.tile_pool(name="pt", bufs=4) as ptp,
            tc.tile_pool(name="o2t", bufs=4) as o2tp,
            tc.tile_pool(name="ob", bufs=3) as obp,
            tc.tile_pool(name="rc", bufs=3) as rcp,
            tc.tile_pool(name="pss", bufs=2, space="PSUM") as pss,
            tc.tile_pool(name="psq", bufs=2, space="PSUM") as psq,
        ):
            # --- weights / constants ---
            wq_sb = wp.tile([128, NDB, 256], F32R, tag="wq")
            wk_sb = wp.tile([128, NDB, 256], F32R, tag="wk")
            wv_sb = wp.tile([128, NDB, 256], F32R, tag="wv")
            wo_sb = wp.tile([128, 8, D], F32R, tag="wo")
            mk_sb = wp.tile([128, 4, 512], F32R, tag="mk")
            ones_sb = wp.tile([1, 64], F32R, tag="ones")
            nc.sync.dma_start(wq_sb[:], wq_d)
            nc.sync.dma_start(wk_sb[:], wk_d)
            nc.sync.dma_start(wv_sb[:], wv_d)
            nc.sync.dma_start(wo_sb[:], wo_d)
            nc.sync.dma_start(mk_sb[:], mk_d)
            ones_f32 = wp.tile([128, 64], F32, tag="ones_f32")
            nc.vector.memset(ones_f32[:], 1.0)
            nc.vector.tensor_copy(ones_sb[:], ones_f32[0:1, :])

            # persistent Q^T / K^T / V(+ones column)
            qt = qkvp.tile([128, 2, N], F32R, tag="qt")  # [(2h)*64e, pair, n]
            kt = qkvp.tile([128, 2, N], F32R, tag="kt")
            vt = qkvp.tile([128, NKB, HPC, 65], F32R, tag="vt")  # [k, kb, h, e|1]
            nc.vector.tensor_copy(
                vt[:, :, :, 64],
                ones_f32[:, 0 : NKB * HPC].rearrange("p (a b) -> p a b", a=NKB),
            )

            # --- phase 1: QKV projections, in two n-halves ---
            for nh in range(2):
                xts = []
                for do in range(NDB):
                    xtile = xp.tile([128, 1024], F32R, tag="xt")
                    nc.sync.dma_start(
                        xtile[:],
                        xt_d[do * 128 : (do + 1) * 128, nh * 1024 : (nh + 1) * 1024],
                    )
                    xts.append(xtile)
                # Q^T and K^T: out [128=(2 heads x 64e), 512 n]
                for w_sb, dst in ((wq_sb, qt), (wk_sb, kt)):
                    for mg in range(2):
                        for qbl in range(2):
                            qb = nh * 2 + qbl
                            ps = pss.tile([128, 512], F32, tag="sc")
                            for do in range(NDB):
                                nc.tensor.matmul(
                                    ps[:],
                                    w_sb[:, do, mg * 128 : (mg + 1) * 128],
                                    xts[do][:, qbl * 512 : (qbl + 1) * 512],
                                    start=(do == 0),
                                    stop=(do == NDB - 1),
                                )
                            nc.vector.tensor_copy(
                                dst[:, mg, qb * 512 : (qb + 1) * 512], ps[:]
                            )
                # V: out [128 k, 256=(4 heads x 64e)]
                for kbl in range(8):
                    kb = nh * 8 + kbl
                    ps = pss.tile([128, 512], F32, tag="sc")
                    nps = ps[:, 0:256]
                    for do in range(NDB):
                        nc.tensor.matmul(
                            nps,
                            xts[do][:, kbl * 128 : (kbl + 1) * 128],
                            wv_sb[:, do, :],
                            start=(do == 0),
                            stop=(do == NDB - 1),
                        )
                    nc.vector.tensor_copy(
                        vt[:, kb, :, 0:64], nps.rearrange("p (h e) -> p h e", h=HPC)
                    )

            # --- phase 2: attention + out-proj, per head pair ---
            for pr in range(2):
                o2t_h = [o2tp.tile([128, N // 2], F32R, tag="o2", name=f"o2t_{pr}_{i}") for i in range(2)]
                for qb in range(NQB):
                    nkb = 4 * qb + 4  # kept key blocks (causal)
                    o_ps = [pss.tile([128, 512], F32, tag="ov", name=f"ov_{pr}_{qb}_{i}") for i in range(2)]
                    for kb in range(nkb):
                        # S^T duo: both heads of the pair for this key block
                        qd = psq.tile([128, 2, 512], F32, tag="qd")
                        for h in range(2):
                            nc.tensor.matmul(
                                qd[:, h, :],
                                kt[64 * h : 64 * (h + 1), pr, kb * 128 : (kb + 1) * 128],
                                qt[64 * h : 64 * (h + 1), pr, qb * 512 : (qb + 1) * 512],
                                start=True,
                                stop=True,
                            )
                        pt_t = ptp.tile([128, 2, 512], F32R, tag="pt")
                        nc.scalar.activation(pt_t[:], qd[:], EXP, scale=SCALE)
                        dg = kb - 4 * qb  # diagonal mask pattern (0..3) if >= 0
                        if dg >= 0:
                            for h in range(2):
                                nc.vector.tensor_mul(
                                    pt_t[:, h, :], pt_t[:, h, :], mk_sb[:, dg, :]
                                )
                        for h in range(2):
                            nc.tensor.matmul(
                                o_ps[h][0:65, :],
                                vt[:, kb, 2 * pr + h, :],
                                pt_t[:, h, :],
                                start=(kb == 0),
                                stop=(kb == nkb - 1),
                            )
                    # normalize into O2T [ (n%2)*64+e , n//2 ].  Bounce O~ to
                    # SBUF first so the PSUM slot frees immediately (the next
                    # q-block's PV otherwise stalls the PE behind normalize).
                    for h in range(2):
                        osb = rcp.tile([65, 512], F32, tag="osb")
                        nc.vector.tensor_copy(osb[:], o_ps[h][0:65, :])
                        rec = rcp.tile([1, 512], F32, tag="rec")
                        nc.vector.reciprocal(rec[:], osb[64:65, :])
                        brs = rcp.tile([64, 512], F32, tag="brs")
                        nc.gpsimd.partition_broadcast(brs[:], rec[:])
                        for par in range(2):
                            nc.vector.tensor_mul(
                                o2t_h[h][
                                    64 * par : 64 * par + 64,
                                    256 * qb : 256 * (qb + 1),
                                ],
                                osb[0:64, par::2],
                                brs[:, par::2],
                            )
                # out-projection: out rows for head hl = 2*pr + h
                for h in range(2):
                    hl = 2 * pr + h
                    for oh in range(2):
                        op = pss.tile([128, 512], F32, tag="sc")
                        for m in range(8):
                            nc.tensor.matmul(
                                op[:],
                                o2t_h[h][:, m::8],
                                wo_sb[:, m, oh * 512 : (oh + 1) * 512],
                                start=(m == 0),
                                stop=(m == 7),
                            )
                        osb = obp.tile([128, 512], F32, tag="ob")
                        nc.vector.tensor_copy(osb[:], op[:])
                        nc.sync.dma_start(
                            out_d[hl * 128 : (hl + 1) * 128, oh * 512 : (oh + 1) * 512],
                            osb[:],
                        )

    nc.compile()
    return nc


def _get_nc():
    if "nc" not in _CACHE:
        _CACHE["nc"] = build_nc()
    return _CACHE["nc"]


def _prep_w(wg):
    """(4, 64, 1024) per-head weights -> [128, 8, 256] SBUF lhsT layout."""
    # WT[d, f=(h*64+e)] = wg[h, e, d]; block d = do*128 + p -> [p, do, f]
    wt = wg.transpose(2, 0, 1).reshape(D, 256)
    return np.ascontiguousarray(wt.reshape(NDB, 128, 256).transpose(1, 0, 2))


def _prep_wo(wot):
    """WoT (1024, 1024) [c, o] -> [128, 8, 1024] with c = 128*m + p."""
    return np.ascontiguousarray(wot.reshape(8, 128, D).transpose(1, 0, 2))


def make_in_maps(x, Wq_lb, Wk_lb, Wv_lb, Wq_la, Wk_la, Wv_la, Wo):
    B = x.shape[0]
    xf = np.asarray(x, np.float32).reshape(B, N, D)
    wot = np.ascontiguousarray(np.asarray(Wo, np.float32).T)  # [c, o]
    wot_rev = np.ascontiguousarray(wot.reshape(16, 64, D)[::-1].reshape(D, D))
    wo_maps = {False: _prep_wo(wot), True: _prep_wo(wot_rev)}

    kp = np.arange(128, dtype=np.int64)[:, None]
    qf = np.arange(512, dtype=np.int64)[None, :]
    mk = np.stack(
        [(qf >= kp + 128 * dg).astype(np.float32) for dg in range(4)], axis=0
    )
    mk = np.ascontiguousarray(mk.transpose(1, 0, 2))  # [128, 4, 512]

    xts = {}
    for b in range(B):
        xts[(b, False)] = np.ascontiguousarray(xf[b].T)
        xts[(b, True)] = np.ascontiguousarray(xf[b][::-1].T)

    wsel = {
        False: (np.asarray(Wq_lb, np.float32), np.asarray(Wk_lb, np.float32),
                np.asarray(Wv_lb, np.float32)),
        True: (np.asarray(Wq_la, np.float32), np.asarray(Wk_la, np.float32),
               np.asarray(Wv_la, np.float32)),
    }
    wcache = {}
    in_maps = []
    for c in range(8):
        b, grp = divmod(c, 4)
        la = grp >= 2
        half = grp % 2
        key = (la, half)
        if key not in wcache:
            wq, wk, wv = wsel[la]
            sl = slice(half * 4, half * 4 + 4)
            wcache[key] = (_prep_w(wq[sl]), _prep_w(wk[sl]), _prep_w(wv[sl]))
        pwq, pwk, pwv = wcache[key]
        in_maps.append(
            {
                "xt": xts[(b, la)],
                "wq": pwq,
                "wk": pwk,
                "wv": pwv,
                "wo": wo_maps[la],
                "mk": mk,
            }
        )
    return in_maps


def assemble(results, B=2):
    out = np.empty((B, N, D), np.float32)
    for c in range(8):
        b, grp = divmod(c, 4)
        part = np.asarray(results[c]["out"])  # (512, 1024)
        if grp >= 2:  # lookahead: un-reverse rows within each head block
            part = part.reshape(HPC, 128, D)[:, ::-1].reshape(512, D)
        out[b, grp * 512 : (grp + 1) * 512] = part
    return out


def kernel(x, Wq_lb, Wk_lb, Wv_lb, Wq_la, Wk_la, Wv_la, Wo):
    nc = _get_nc()
    in_maps = make_in_maps(x, Wq_lb, Wk_lb, Wv_lb, Wq_la, Wk_la, Wv_la, Wo)
    res = run_bass_kernel_spmd(nc, in_maps, list(range(8)))
    B, T, F_, D_ = x.shape
    return assemble(res.results, B).reshape(B, T, F_, D_)


# revision 22
# speedup vs baseline: 1.2398x; 1.0444x over previous
"""Trainium2 Bass kernel for nn_BidirectionalTemporalAttention.

Reference computation (B=2, T=16, F=128, D=1024, N=T*F=2048):
  xf = x.reshape(B, N, D)
  lookback branch: 8 heads, E=64, causal mask (keep k <= q)
  lookahead branch: 8 heads, anti-causal (keep k >= q)
  o = concat([o_lb, o_la], heads) -> (B, 16, N, 64) -> RAW reshape (B, N, D)
  out = o @ Wo^T -> (B, T, F, D)

The raw reshape means out row r = h*128 + g depends only on head h (tokens
16g..16g+15 of that head).  So with 4 heads per core each core's 512 output
rows are fully local: no collectives, the host just concatenates row slices.

Sharding over 8 cores: (batch b in 2) x (group in [lb0-3, lb4-7, la0-3, la4-7]).
Lookahead cores receive the token-reversed sequence so one SPMD causal program
serves all cores; their outputs are un-reversed on the host (row reversal
within each 128-row head block, plus a j-group reversal folded into Wo).

Per-core kernel layout choices:
  - S^T blocks [k(128 part), q(512 free)] so softmax-denominator and PV both
    contract over k on the partition axis.
  - exp has no max-subtraction (scores are O(10), safe in fp32); softmax
    denominator comes free as a ones-column appended to V in the PV matmul.
  - attention output written into O2T [128=(n%2)*64+e, n//2]; its strided
    views O2T[:, m::8] are exactly the K=128 lhsT tiles the out-projection
    needs under the reference's raw (H,N,E)->(N,D) reshape.
  - all matmul operands are float32r (1 cycle/row on the PE at free>=256,
    ~1.5e-4 relative rounding vs 4 cycles/row for exact fp32).
"""

import sys

if "/opt/trn_rl_repo" not in sys.path:
    sys.path.insert(0, "/opt/trn_rl_repo")

import numpy as np

import concourse.bass as bass  # noqa: F401
import concourse.mybir as mybir
import concourse.tile as tile
from concourse import bacc
from concourse import bass_utils as _bu
from concourse.bass_utils import run_bass_kernel_spmd

# LDWEIGHTS of fp32 weights serializes with matmuls and costs ~35% of PE time
# on this kernel; walrus's ldw optimization (background weight-buffer loads)
# is disabled by default in this toolchain. Flip just that flag.
_ENABLE_LDW_OPT = False
_orig_run_command = _bu.run_command


def _patched_run_command(cmd, *a, **kw):
    if _ENABLE_LDW_OPT and isinstance(cmd, list):
        cmd = [
            "--enable-ldw-opt=true" if c == "--enable-ldw-opt=false" else c
            for c in cmd
        ]
    return _orig_run_command(cmd, *a, **kw)


_bu.run_command = _patched_run_command

F32 = mybir.dt.float32
F32R = mybir.dt.float32r
EXP = mybir.ActivationFunctionType.Exp

N = 2048  # tokens per batch
D = 1024  # embed dim
E = 64  # head dim
HPC = 4  # heads per core
NQB = 4  # query blocks of 512
NKB = 16  # key blocks of 128
NDB = 8  # d blocks of 128
SCALE = 0.125  # 1/sqrt(E)

_CACHE = {}


def build_nc():
    nc = bacc.Bacc("TRN2", target_bir_lowering=False, debug=False)

    xt_d = nc.dram_tensor("xt", [D, N], F32R, kind="ExternalInput").ap()
    wq_d = nc.dram_tensor("wq", [128, NDB, 256], F32R, kind="ExternalInput").ap()
    wk_d = nc.dram_tensor("wk", [128, NDB, 256], F32R, kind="ExternalInput").ap()
    wv_d = nc.dram_tensor("wv", [128, NDB, 256], F32R, kind="ExternalInput").ap()
    wo_d = nc.dram_tensor("wo", [128, 8, D], F32R, kind="ExternalInput").ap()
    mk_d = nc.dram_tensor("mk", [128, 4, 512], F32R, kind="ExternalInput").ap()
    out_d = nc.dram_tensor("out", [512, D], F32, kind="ExternalOutput").ap()

    with tile.TileContext(nc) as tc:
        with (
            tc.tile_pool(name="w", bufs=1) as wp,
            tc.tile_pool(name="xp", bufs=8) as xp,
            tc.tile_pool(name="qkv", bufs=1) as qkvp,
            tc.tile_pool(name="pt", bufs=3) as ptp,
            tc.tile_pool(name="o2t", bufs=4) as o2tp,
            tc.tile_pool(name="ob", bufs=2) as obp,
            tc.tile_pool(name="rc", bufs=2) as rcp,
            tc.tile_pool(name="pss", bufs=2, space="PSUM") as pss,
            tc.tile_pool(name="psq", bufs=2, space="PSUM") as psq,
        ):
            # --- weights / constants (wq + x first: they gate the first MMs;
            #     wo/mk are deferred until phase 2 so they don't queue ahead) ---
            wq_sb = wp.tile([128, NDB, 256], F32R, tag="wq")
            wk_sb = wp.tile([128, NDB, 256], F32R, tag="wk")
            wv_sb = wp.tile([128, NDB, 256], F32R, tag="wv")
            mk_sb = wp.tile([128, 4, 512], F32R, tag="mk")
            nc.sync.dma_start(wq_sb[:], wq_d)
            xts = []
            for do in range(NDB):
                xtile = xp.tile([128, N], F32R, tag="xt", name=f"xt_{do}")
                nc.sync.dma_start(xtile[:], xt_d[do * 128 : (do + 1) * 128, :])
                xts.append(xtile)
            nc.sync.dma_start(wk_sb[:], wk_d)
            nc.sync.dma_start(wv_sb[:], wv_d)
            ones_f32 = wp.tile([128, 64], F32, tag="ones_f32")
            nc.vector.memset(ones_f32[:], 1.0)

            # persistent Q^T / K^T / V(+ones column)
            qt = qkvp.tile([128, 2, N], F32R, tag="qt")  # [(2h)*64e, pair, n]
            kt = qkvp.tile([128, 2, N], F32R, tag="kt")
            vt = qkvp.tile([128, NKB, HPC, 65], F32R, tag="vt")  # [k, kb, h, e|1]
            nc.vector.tensor_copy(
                vt[:, :, :, 64],
                ones_f32[:, 0 : NKB * HPC].rearrange("p (a b) -> p a b", a=NKB),
            )

            # --- phase 1: QKV projections ---
            # Q^T and K^T: out [128=(2 heads x 64e), 512 n]
            for w_sb, dst in ((wq_sb, qt), (wk_sb, kt)):
                for mg in range(2):
                    for qb in range(NQB):
                        ps = pss.tile([128, 512], F32, tag="sc")
                        for do in range(NDB):
                            nc.tensor.matmul(
                                ps[:],
                                w_sb[:, do, mg * 128 : (mg + 1) * 128],
                                xts[do][:, qb * 512 : (qb + 1) * 512],
                                start=(do == 0),
                                stop=(do == NDB - 1),
                            )
                        nc.vector.tensor_copy(
                            dst[:, mg, qb * 512 : (qb + 1) * 512], ps[:]
                        )
            # V: out [128 k, 256=(4 heads x 64e)]
            for kb in range(NKB):
                ps = pss.tile([128, 512], F32, tag="sc")
                nps = ps[:, 0:256]
                for do in range(NDB):
                    nc.tensor.matmul(
                        nps,
                        xts[do][:, kb * 128 : (kb + 1) * 128],
                        wv_sb[:, do, :],
                        start=(do == 0),
                        stop=(do == NDB - 1),
                    )
                nc.vector.tensor_copy(
                    vt[:, kb, :, 0:64], nps.rearrange("p (h e) -> p h e", h=HPC)
                )

            # weights needed from phase 2 on; Wo reuses the qkv-weight slots
            # (dead after phase 1) via matching pool tags
            nc.sync.dma_start(mk_sb[:], mk_d)
            wo_parts = []
            for i, tg in enumerate(("wq", "wk", "wv", "wo2")):
                wpart = wp.tile([128, 2, D], F32R, tag=tg, name=f"wo_{i}")
                nc.sync.dma_start(wpart[:], wo_d[:, 2 * i : 2 * i + 2, :])
                wo_parts.append(wpart)

            # --- phase 2: attention for both pairs, then both out-projs
            #     (out-proj emitted after the other pair's attention so the
            #      in-order PE stream never head-of-line blocks on normalize) ---
            o2t_all = {}
            for pr in range(2):
                o2t_h = [o2tp.tile([128, N // 2], F32R, tag="o2", name=f"o2t_{pr}_{i}") for i in range(2)]
                o2t_all[pr] = o2t_h
                for qb in range(NQB):
                    nkb = 4 * qb + 4  # kept key blocks (causal)
                    o_ps = [pss.tile([128, 512], F32, tag="ov", name=f"ov_{pr}_{qb}_{i}") for i in range(2)]
                    for kb in range(nkb):
                        # S^T duo: both heads of the pair for this key block
                        qd = psq.tile([128, 2, 512], F32, tag="qd")
                        for h in range(2):
                            nc.tensor.matmul(
                                qd[:, h, :],
                                kt[64 * h : 64 * (h + 1), pr, kb * 128 : (kb + 1) * 128],
                                qt[64 * h : 64 * (h + 1), pr, qb * 512 : (qb + 1) * 512],
                                start=True,
                                stop=True,
                            )
                        pt_t = ptp.tile([128, 2, 512], F32R, tag="pt")
                        nc.scalar.activation(pt_t[:], qd[:], EXP, scale=SCALE)
                        dg = kb - 4 * qb  # diagonal mask pattern (0..3) if >= 0
                        if dg >= 0:
                            for h in range(2):
                                nc.vector.tensor_mul(
                                    pt_t[:, h, :], pt_t[:, h, :], mk_sb[:, dg, :]
                                )
                        for h in range(2):
                            nc.tensor.matmul(
                                o_ps[h][0:65, :],
                                vt[:, kb, 2 * pr + h, :],
                                pt_t[:, h, :],
                                start=(kb == 0),
                                stop=(kb == nkb - 1),
                            )
                    # normalize into O2T [ (n%2)*64+e , n//2 ].  Bounce O~ to
                    # SBUF first so the PSUM slot frees immediately (the next
                    # q-block's PV otherwise stalls the PE behind normalize).
                    for h in range(2):
                        osb = rcp.tile([65, 512], F32, tag="osb")
                        nc.vector.tensor_copy(osb[:], o_ps[h][0:65, :])
                        rec = rcp.tile([1, 512], F32, tag="rec")
                        nc.vector.reciprocal(rec[:], osb[64:65, :])
                        brs = rcp.tile([64, 512], F32, tag="brs")
                        nc.gpsimd.partition_broadcast(brs[:], rec[:])
                        for par in range(2):
                            nc.vector.tensor_mul(
                                o2t_h[h][
                                    64 * par : 64 * par + 64,
                                    256 * qb : 256 * (qb + 1),
                                ],
                                osb[0:64, par::2],
                                brs[:, par::2],
                            )
            # out-projection: out rows for head hl = 2*pr + h
            for pr in range(2):
                for h in range(2):
                    hl = 2 * pr + h
                    for oh in range(2):
                        op = pss.tile([128, 512], F32, tag="sc")
                        for m in range(8):
                            nc.tensor.matmul(
                                op[:],
                                o2t_all[pr][h][:, m::8],
                                wo_parts[m // 2][:, m % 2, oh * 512 : (oh + 1) * 512],
                                start=(m == 0),
                                stop=(m == 7),
                            )
                        osb = obp.tile([128, 512], F32, tag="ob")
                        nc.vector.tensor_copy(osb[:], op[:])
                        nc.sync.dma_start(
                            out_d[hl * 128 : (hl + 1) * 128, oh * 512 : (oh + 1) * 512],
                            osb[:],
                        )

    nc.compile()
    return nc


def _get_nc():
    if "nc" not in _CACHE:
        _CACHE["nc"] = build_nc()
    return _CACHE["nc"]


def _prep_w(wg):
    """(4, 64, 1024) per-head weights -> [128, 8, 256] SBUF lhsT layout."""
    # WT[d, f=(h*64+e)] = wg[h, e, d]; block d = do*128 + p -> [p, do, f]
    wt = wg.transpose(2, 0, 1).reshape(D, 256)
    return np.ascontiguousarray(wt.reshape(NDB, 128, 256).transpose(1, 0, 2))


def _prep_wo(wot):
    """WoT (1024, 1024) [c, o] -> [128, 8, 1024] with c = 128*m + p."""
    return np.ascontiguousarray(wot.reshape(8, 128, D).transpose(1, 0, 2))


def make_in_maps(x, Wq_lb, Wk_lb, Wv_lb, Wq_la, Wk_la, Wv_la, Wo):
    B = x.shape[0]
    xf = np.asarray(x, np.float32).reshape(B, N, D)
    wot = np.ascontiguousarray(np.asarray(Wo, np.float32).T)  # [c, o]
    wot_rev = np.ascontiguousarray(wot.reshape(16, 64, D)[::-1].reshape(D, D))
    wo_maps = {False: _prep_wo(wot), True: _prep_wo(wot_rev)}

    kp = np.arange(128, dtype=np.int64)[:, None]
    qf = np.arange(512, dtype=np.int64)[None, :]
    mk = np.stack(
        [(qf >= kp + 128 * dg).astype(np.float32) for dg in range(4)], axis=0
    )
    mk = np.ascontiguousarray(mk.transpose(1, 0, 2))  # [128, 4, 512]

    xts = {}
    for b in range(B):
        xts[(b, False)] = np.ascontiguousarray(xf[b].T)
        xts[(b, True)] = np.ascontiguousarray(xf[b][::-1].T)

    wsel = {
        False: (np.asarray(Wq_lb, np.float32), np.asarray(Wk_lb, np.float32),
                np.asarray(Wv_lb, np.float32)),
        True: (np.asarray(Wq_la, np.float32), np.asarray(Wk_la, np.float32),
               np.asarray(Wv_la, np.float32)),
    }
    wcache = {}
    in_maps = []
    for c in range(8):
        b, grp = divmod(c, 4)
        la = grp >= 2
        half = grp % 2
        key = (la, half)
        if key not in wcache:
            wq, wk, wv = wsel[la]
            sl = slice(half * 4, half * 4 + 4)
            wcache[key] = (_prep_w(wq[sl]), _prep_w(wk[sl]), _prep_w(wv[sl]))
        pwq, pwk, pwv = wcache[key]
        in_maps.append(
            {
                "xt": xts[(b, la)],
                "wq": pwq,
                "wk": pwk,
                "wv": pwv,
                "wo": wo_maps[la],
                "mk": mk,
            }
        )
    return in_maps


def assemble(results, B=2):
    out = np.empty((B, N, D), np.float32)
    for c in range(8):
        b, grp = divmod(c, 4)
        part = np.asarray(results[c]["out"])  # (512, 1024)
        if grp >= 2:  # lookahead: un-reverse rows within each head block
            part = part.reshape(HPC, 128, D)[:, ::-1].reshape(512, D)
        out[b, grp * 512 : (grp + 1) * 512] = part
    return out


def kernel(x, Wq_lb, Wk_lb, Wv_lb, Wq_la, Wk_la, Wv_la, Wo):
    nc = _get_nc()
    in_maps = make_in_maps(x, Wq_lb, Wk_lb, Wv_lb, Wq_la, Wk_la, Wv_la, Wo)
    res = run_bass_kernel_spmd(nc, in_maps, list(range(8)))
    B, T, F_, D_ = x.shape
    return assemble(res.results, B).reshape(B, T, F_, D_)
